# revision 10
# baseline (speedup 1.0000x reference)
"""GCN (2x GCNConv + global mean pool + FC) on 8 Trainium2 NeuronCores.

Strategy (graph-parallel, dst-sharded, aggregate-then-transform):
  - Nodes sharded contiguously across 8 cores (12.5K nodes/core, padded to
    12544 = 98 blocks of 128).
  - Layer tables hold PRE-transform rows scaled by dinv[src]: tbl1 = x*dinv
    (host-prescaled, uploaded FULL per core in gather layout -> no phase-0
    compute and no tbl1 collectives), tbl2 = relu(agg1@W1*dinv+b1)*dinv.
    W is applied AFTER aggregation (linearity), so both tables are 128-wide.
  - tbl2 is AllGather'd in 10 groups (9x10 + 1x8 blocks, core-major rows per
    group) pipelined with production; int16 quarters = 3 groups each (30720
    rows) + group 9, with a memset zero row in each quarter's tail gap.
  - Per-edge gather: gpsimd.dma_gather (int16 idxs within quarter), streams
    bucketed by (src-quarter q, dst-4-block-group).  HYBRID layout: per
    (q<3, dst-block) the first 4 copies of each dst slot sit at their slot
    position in 4 "aligned" tiles (pad gathers the zero row), leftovers and
    all of q3 go to dense tiles.
  - Aggregation TRANSPOSED on the PE: psum[f, slot] += gb[e,f]^T M[e,slot];
    M = identity (constant) for aligned tiles, a one-hot S for dense tiles.
    S matrices are built 8-at-a-time on the DVE via zero-stride broadcast
    APs (one tensor_tensor is_equal per 8 dense tiles).
  - Self-loops: psum[f, slot] += selfrow[p,f]^T I[p,slot].
  - Drain: ACT copy -> PE matmul(W) -> (x dinv + b) -> relu -> table / pool.
  - Pool: one-hot graph-membership matmul per node-block, accumulated in
    SBUF; final mean + FC on host (tiny: G=64 x H2=64).
"""
import os
os.environ.setdefault("JAX_PLATFORMS", "cpu")
import sys
if "/opt/trn_rl_repo" not in sys.path:
    sys.path.insert(0, "/opt/trn_rl_repo")
import time
from contextlib import ExitStack

import numpy as np
import ml_dtypes

import concourse.bacc as bacc
import concourse.bass as bass
import concourse.tile as tile
import concourse.mybir as mybir
from concourse.bass_utils import run_bass_kernel_spmd
from concourse.library_config import mlp

NCORES = 8
F = 128          # feature width of both tables (x and h1)
H2 = 64          # hidden-2 feature width
G = 64           # graphs
QROWS = 32768    # int16 quarter size (rows)
QN = 4
DST_GB = 4       # dst blocks per gather-stream group
K = 8            # S matrices built per DVE op
ALIGN_T = 4      # aligned tiles per (q<3, dst block)
ALLOC_ROWS = QN * QROWS

bf16 = mybir.dt.bfloat16
f32 = mybir.dt.float32
i16 = mybir.dt.int16

LAST_EXEC_S = None
LAST_PREP_S = None
LAST_COMPILE_S = None
LAST_RESULTS = None

_BUILD_CACHE = {}


def _ceil(a, b):
    return -(-a // b)


# ---- table layouts (npcp = 12544 = 98 blocks) ----
# tbl1 (input x, no collective): quarter-core-major.  Quarter q<3 holds
# local rows [q*3840, (q+1)*3840) of every core; q3 holds [11520, 12544).
T1_QL0 = [0, 3840, 7680, 11520]
T1_QRPC = [3840, 3840, 3840, 1024]
# tbl2 (AllGather per group): 9 groups of 1280 rows + 1 of 1024, core-major
# per group.  Quarter q<3 = groups [3q, 3q+3) at offsets {0,10240,20480}.
T2_GL0 = [1280 * i for i in range(9)] + [11520]
T2_GRPC = [1280] * 9 + [1024]
T2_GOFF = [0, 10240, 20480, 32768, 43008, 53248, 65536, 75776, 86016, 98304]
ZROW = [30720, 30720, 30720, 8192]   # zero-row idx within each quarter


def _make_groups(nb):
    groups = []
    b = 0
    while b < nb:
        groups.append(list(range(b, min(b + DST_GB, nb))))
        b += DST_GB
    return groups


class Sched:
    """Hybrid stream schedule (shared across cores).

    Per (q, dst-group sg): ALIGN_T aligned tiles per block (q<3) followed
    by dense tiles holding per-(q,b) leftover buckets (capacity = max over
    cores), padded to x128 at the (q,sg) end."""

    def __init__(self, dcnt, nb, groups):
        self.nb, self.groups = nb, groups
        cap = dcnt.max(axis=0).astype(np.int64)      # [QN, nb] dense caps
        self.cap = cap
        ng = len(groups)
        self.aq = [ALIGN_T, ALIGN_T, ALIGN_T, 0]
        self.ct = np.zeros((QN, ng), np.int64)       # tiles per call
        self.gstart = np.zeros((QN, ng), np.int64)   # stream offset (edges)
        self.boff = np.zeros((QN, nb), np.int64)     # dense block offset
        run = 0
        for g, blocks in enumerate(groups):
            for q in range(QN):
                self.gstart[q, g] = run
                na = self.aq[q] * len(blocks)
                off = na * 128
                for b in blocks:
                    self.boff[q, b] = run + off
                    off += cap[q, b]
                ctg = na + _ceil(off - na * 128, 128)
                self.ct[q, g] = ctg
                run += ctg * 128
        self.ntot = int(run)
        self.nt = self.ntot // 128
        # dense instances: per (g, b): list of (q, t_local, iid)
        self.inst = {}
        iid = 0
        for g, blocks in enumerate(groups):
            for b in blocks:
                lst = []
                for q in range(QN):
                    c = int(cap[q, b])
                    if c == 0:
                        continue
                    lo = int(self.boff[q, b] - self.gstart[q, g])
                    hi = lo + c
                    for t in range(lo // 128, _ceil(hi, 128)):
                        lst.append((q, t, iid))
                        iid += 1
                self.inst[(g, b)] = lst
        self.ninst = iid

    def key(self):
        return (self.ntot, self.ninst,
                tuple(self.cap.reshape(-1).tolist()))


def _schedule(src, dst, npc, nb):
    """Returns sched, gidx1, gidx2 [NCORES, ntot] i16, cols [NCORES,128,ninst]."""
    cd = dst // npc
    dl = dst % npc
    blk = dl >> 7
    slot = dl & 127
    cs = src // npc
    l = src % npc
    q = np.minimum(l // 3840, 3)
    idx1 = (cs * np.take(T1_QRPC, q) + (l - np.take(T1_QL0, q))).astype(np.int16)
    gi = np.minimum(l // 1280, 9)
    pos2 = (np.take(T2_GOFF, gi) + cs * np.take(T2_GRPC, gi)
            + (l - np.take(T2_GL0, gi)))
    idx2 = (pos2 - q * QROWS).astype(np.int16)

    key = ((cd * QN + q) * nb + blk).astype(np.int64)
    order = np.lexsort((slot, key))
    ks = key[order]
    ss = slot[order]
    i1s = idx1[order]
    i2s = idx2[order]
    n = len(ks)

    # rank within (bucket, slot)
    grp = ks * 128 + ss
    newg = np.empty(n, bool); newg[0] = True
    newg[1:] = grp[1:] != grp[:-1]
    gfirst = np.maximum.accumulate(np.where(newg, np.arange(n), 0))
    rank = np.arange(n) - gfirst

    qs = (ks // nb) % QN
    aq_s = np.where(qs < 3, ALIGN_T, 0)
    aligned = rank < aq_s

    # dense rank within bucket
    didx = np.nonzero(~aligned)[0]
    dk = ks[didx]
    newk = np.empty(len(dk), bool)
    if len(dk):
        newk[0] = True
        newk[1:] = dk[1:] != dk[:-1]
    dfirst = np.maximum.accumulate(np.where(newk, np.arange(len(dk)), 0))
    drank = np.arange(len(dk)) - dfirst

    nbuck = NCORES * QN * nb
    dcnt = np.bincount(dk, minlength=nbuck).reshape(NCORES, QN, nb)
    sched = Sched(dcnt, nb, _make_groups(nb))

    # stream positions
    sg_s = blk[order] // DST_GB
    bi_s = blk[order] - sg_s * DST_GB
    gst = sched.gstart[qs, sg_s]
    pos = np.empty(n, np.int64)
    am = aligned
    pos[am] = gst[am] + (bi_s[am] * ALIGN_T + rank[am]) * 128 + ss[am]
    pos[didx] = sched.boff.reshape(-1)[dk % (QN * nb)] + drank

    # default idx = quarter zero row; slot = -1
    zr1 = np.zeros(sched.ntot, np.int16)
    for g in range(len(sched.groups)):
        for qq in range(QN):
            s0 = int(sched.gstart[qq, g])
            s1 = s0 + int(sched.ct[qq, g]) * 128
            zr1[s0:s1] = ZROW[qq]
    gidx1 = np.broadcast_to(zr1, (NCORES, sched.ntot)).copy()
    gidx2 = gidx1.copy()
    slot_all = np.full((NCORES, sched.ntot), -1.0, np.float32)
    core_s = ks // (QN * nb)
    gidx1[core_s, pos] = i1s
    gidx2[core_s, pos] = i2s
    slot_all[core_s[didx], pos[didx]] = ss[didx].astype(np.float32)

    # per-dense-instance slot columns
    cols = np.full((NCORES, 128, sched.ninst), -1.0, np.float32)
    for g, blocks in enumerate(sched.groups):
        for b in blocks:
            for qq, t, iid in sched.inst[(g, b)]:
                base = int(sched.gstart[qq, g]) + t * 128
                lo = int(sched.boff[qq, b])
                hi = lo + int(sched.cap[qq, b])
                p = np.arange(base, base + 128)
                m = (p >= lo) & (p < hi)
                cols[:, :, iid] = np.where(
                    m, slot_all[:, base : base + 128], -1.0)
    return sched, gidx1, gidx2, cols


def _wrap_idx(sched, gidx):
    """[NCORES, 128, nt*8] wrapped (x16) + replicated (x8) idx layout."""
    nt = sched.nt
    out = np.zeros((NCORES, 128, nt * 8), np.int16)
    calls = []
    for g in range(len(sched.groups)):
        for q in range(QN):
            if sched.ct[q, g]:
                calls.append((int(sched.gstart[q, g]) // 128,
                              int(sched.ct[q, g])))
    for c in range(NCORES):
        for s0, ln in calls:
            seg = gidx[c, s0 * 128 : (s0 + ln) * 128]
            w = seg.reshape(-1, 16).T
            out[c, :, s0 * 8 : (s0 + ln) * 8] = np.tile(w, (8, 1))
    return out


def _build_bass(npcp, nb, sched, ninst_pad, maxct, g_graphs):
    groups = sched.groups
    nt = sched.nt

    nc = bacc.Bacc("TRN2", num_devices=NCORES, num_swdge_queues=4,
                   dynamic_dma_scratch_size=65536)
    xtbl = nc.dram_tensor("xtbl", [ALLOC_ROWS, F], bf16, kind="ExternalInput")
    xself = nc.dram_tensor("xself", [npcp, F], bf16, kind="ExternalInput")
    idx1_d = nc.dram_tensor("idx1", [128, nt * 8], i16, kind="ExternalInput")
    idx2_d = nc.dram_tensor("idx2", [128, nt * 8], i16, kind="ExternalInput")
    slots_d = nc.dram_tensor("slots", [128, ninst_pad], bf16, kind="ExternalInput")
    dinv_d = nc.dram_tensor("dinvc", [128, nb], f32, kind="ExternalInput")
    batch_d = nc.dram_tensor("batchc", [128, nb], f32, kind="ExternalInput")
    b1_d = nc.dram_tensor("b1bc", [128, F], f32, kind="ExternalInput")
    iota_d = nc.dram_tensor("iota", [128, 128], bf16, kind="ExternalInput")
    ident_d = nc.dram_tensor("ident", [128, 128], bf16, kind="ExternalInput")
    w1_d = nc.dram_tensor("w1", [F, F], bf16, kind="ExternalInput")
    w2_d = nc.dram_tensor("w2", [F, H2], bf16, kind="ExternalInput")
    out_d = nc.dram_tensor("pooled", [g_graphs, H2], f32, kind="ExternalOutput")

    tbl2_loc = nc.dram_tensor("tbl2_loc", [npcp, F], bf16)
    tbl2_ag = nc.dram_tensor("tbl2_ag", [ALLOC_ROWS, F], bf16,
                             addr_space="Shared")
    rg = [list(range(NCORES))]

    with tile.TileContext(nc) as tc:
        with ExitStack() as ctx:
            cpool = ctx.enter_context(tc.tile_pool(name="const", bufs=1))
            stage = ctx.enter_context(tc.tile_pool(name="stage", bufs=6))
            gbufp = ctx.enter_context(tc.tile_pool(name="gbuf", bufs=8))
            sp = ctx.enter_context(tc.tile_pool(name="sp", bufs=4))
            dr = ctx.enter_context(tc.tile_pool(name="dr", bufs=6))
            psum = ctx.enter_context(
                tc.tile_pool(name="psum", bufs=1, space=bass.MemorySpace.PSUM)
            )

            nc.gpsimd.load_library(mlp)

            # zero rows in tbl2_ag quarter gaps
            zrow = cpool.tile([1, F], bf16)
            nc.vector.memset(zrow[:], 0.0)
            for q in range(QN):
                r = q * QROWS + ZROW[q]
                nc.sync.dma_start(tbl2_ag[r : r + 1, :], zrow[:])

            w1 = cpool.tile([F, F], bf16)
            nc.sync.dma_start(w1[:], w1_d[:])
            w2 = cpool.tile([F, H2], bf16)
            nc.sync.dma_start(w2[:], w2_d[:])
            iota = cpool.tile([128, 128], bf16)
            nc.sync.dma_start(iota[:], iota_d[:])
            ident = cpool.tile([128, 128], bf16)
            nc.sync.dma_start(ident[:], ident_d[:])
            b1bc = cpool.tile([128, F], f32)
            nc.sync.dma_start(b1bc[:], b1_d[:])
            dinvc = cpool.tile([128, nb], f32)
            nc.sync.dma_start(dinvc[:], dinv_d[:])
            batchc = cpool.tile([128, nb], f32)
            nc.sync.dma_start(batchc[:], batch_d[:])
            slotc = cpool.tile([128, ninst_pad], bf16)
            nc.sync.dma_start(slotc[:], slots_d[:])
            idx1_sb = cpool.tile([128, nt * 8], i16)
            nc.sync.dma_start(idx1_sb[:], idx1_d[:])
            idx2_sb = cpool.tile([128, nt * 8], i16)
            nc.sync.dma_start(idx2_sb[:], idx2_d[:])

            pooled_sb = cpool.tile([g_graphs, H2], f32)
            nc.vector.memset(pooled_sb[:], 0.0)

            qcount = [0]

            def run_layer(tbl_src, idx_sb, self_dram, drain_fn, sg_hook=None):
                sbuilt = {}
                for sg, blocks in enumerate(groups):
                    gbs = {}
                    for q in range(QN):
                        ctq = int(sched.ct[q, sg])
                        if ctq == 0:
                            continue
                        ecol = int(sched.gstart[q, sg]) // 16
                        gb = gbufp.tile([128, maxct, 128], bf16, tag="gb")
                        nc.gpsimd.dma_gather(
                            gb[:, 0:ctq, :],
                            tbl_src[q * QROWS : (q + 1) * QROWS, :],
                            idx_sb[:, ecol : ecol + ctq * 8],
                            ctq * 128, ctq * 128, 128,
                            single_packet=False,
                            queue_num=qcount[0] % 4,
                        )
                        qcount[0] += 1
                        gbs[q] = gb
                    for bi, b in enumerate(blocks):
                        insts = sched.inst[(sg, b)]
                        alist = [(q, bi * ALIGN_T + a)
                                 for q in range(3) for a in range(ALIGN_T)]
                        total = len(alist) + len(insts)
                        pm = psum.tile([128, 128], f32, tag="agg", bufs=5)
                        selfrow = dr.tile([128, 128], bf16, tag="selfrow")
                        nc.sync.dma_start(
                            selfrow[:], self_dram[b * 128 : (b + 1) * 128, :]
                        )
                        nc.tensor.matmul(
                            pm[:], selfrow[:], ident[:],
                            start=True, stop=(total == 0),
                        )
                        j = 0
                        for q, t in alist:
                            j += 1
                            nc.tensor.matmul(
                                pm[:], gbs[q][:, t, :], ident[:],
                                start=False, stop=(j == total),
                            )
                        for q, t, iid in insts:
                            kb = iid // K
                            if kb not in sbuilt:
                                S = sp.tile([128, K * 128], bf16, tag="S",
                                            bufs=12)
                                s3 = S[:].rearrange("p (k c) -> p k c", k=K)
                                nc.vector.tensor_tensor(
                                    s3,
                                    iota[:].unsqueeze(1).to_broadcast(
                                        (128, K, 128)),
                                    slotc[:, kb * K : (kb + 1) * K]
                                    .unsqueeze(2).to_broadcast((128, K, 128)),
                                    mybir.AluOpType.is_equal,
                                )
                                sbuilt[kb] = S
                            o = (iid % K) * 128
                            j += 1
                            nc.tensor.matmul(
                                pm[:], gbs[q][:, t, :],
                                sbuilt[kb][:, o : o + 128],
                                start=False, stop=(j == total),
                            )
                        drain_fn(b, pm)
                    if sg_hook is not None:
                        sg_hook(blocks[-1])

            def drain1(b, pm):
                aggT = dr.tile([128, 128], bf16, tag="aggT")
                nc.scalar.activation(
                    aggT[:], pm[:], mybir.ActivationFunctionType.Copy
                )
                pz = psum.tile([128, F], f32, tag="pz", bufs=2)
                nc.tensor.matmul(pz[:], aggT[:], w1[:], start=True, stop=True)
                u = dr.tile([128, F], f32, tag="u")
                nc.vector.scalar_tensor_tensor(
                    u[:], pz[:], dinvc[:, b : b + 1], b1bc[:],
                    mybir.AluOpType.mult, mybir.AluOpType.add,
                )
                hs = stage.tile([128, F], bf16, tag="hs")
                nc.scalar.activation(
                    hs[:], u[:], mybir.ActivationFunctionType.Relu,
                    scale=dinvc[:, b : b + 1],
                )
                nc.sync.dma_start(tbl2_loc[b * 128 : (b + 1) * 128, :], hs[:])

            def drain2(b, pm):
                a2T = dr.tile([128, 128], bf16, tag="aggT")
                nc.scalar.activation(
                    a2T[:], pm[:], mybir.ActivationFunctionType.Copy
                )
                pz = psum.tile([128, H2], f32, tag="pz", bufs=2)
                nc.tensor.matmul(pz[:], a2T[:], w2[:], start=True, stop=True)
                a2 = dr.tile([128, H2], bf16, tag="a2")
                nc.scalar.activation(
                    a2[:], pz[:], mybir.ActivationFunctionType.Copy,
                    scale=dinvc[:, b : b + 1],
                )
                spool = sp.tile([128, g_graphs], bf16, tag="spool")
                nc.vector.tensor_scalar(
                    spool[:], iota[:, 0:g_graphs], batchc[:, b : b + 1],
                    None, mybir.AluOpType.is_equal,
                )
                pp = psum.tile([g_graphs, H2], f32, tag="pp", bufs=1)
                nc.tensor.matmul(pp[:], spool[:], a2[:], start=True, stop=True)
                nc.vector.tensor_add(pooled_sb[:], pooled_sb[:], pp[:])

            # emit each tbl2 AllGather as soon as its last block drains, so
            # the trigger sits early in the in-order gpsimd queue and the
            # collective overlaps layer-1's tail
            agdone = [0]

            def ag_hook(last_block):
                while agdone[0] < 10:
                    gi = agdone[0]
                    lastb = (T2_GL0[gi] + T2_GRPC[gi]) // 128 - 1
                    if lastb > last_block:
                        break
                    l0, rows = T2_GL0[gi], T2_GRPC[gi]
                    off = T2_GOFF[gi]
                    nc.gpsimd.collective_compute(
                        "AllGather", mybir.AluOpType.bypass,
                        replica_groups=rg,
                        ins=[tbl2_loc[l0 : l0 + rows, :]],
                        outs=[tbl2_ag[off : off + rows * NCORES, :]],
                    )
                    agdone[0] += 1

            run_layer(xtbl, idx1_sb, xself, drain1, sg_hook=ag_hook)
            assert agdone[0] == 10
            run_layer(tbl2_ag, idx2_sb, tbl2_loc, drain2)

            nc.sync.dma_start(out_d[:], pooled_sb[:])

    nc.compile()
    return nc


def _install_trace_hooks():
    """Register the axon NTFF profile hook (missing antenv.axon_hooks shim)
    and neuter the artifact upload. Dev/profiling only (K_TRACE_DIR)."""
    import types
    import ctypes
    import contextlib

    if "antenv.axon_hooks" in sys.modules:
        return
    lib = ctypes.CDLL("/opt/axon/libaxon_pjrt.so")
    lib.axon_start_nrt_profile.argtypes = [
        ctypes.POINTER(ctypes.c_int64), ctypes.c_size_t,
    ]
    lib.axon_start_nrt_profile.restype = ctypes.c_int64
    lib.axon_stop_nrt_profile.argtypes = [ctypes.c_char_p]
    lib.axon_stop_nrt_profile.restype = ctypes.c_int64

    @contextlib.contextmanager
    def _hook(output_dir, device_ids):
        import jax
        jax.devices()
        if device_ids:
            ids = (ctypes.c_int64 * len(device_ids))(*device_ids)
            rc = lib.axon_start_nrt_profile(ids, len(device_ids))
        else:
            rc = lib.axon_start_nrt_profile(None, 0)
        if rc != 0:
            raise RuntimeError(f"axon_start_nrt_profile rc={rc}")
        try:
            yield
        finally:
            nfiles = lib.axon_stop_nrt_profile(str(output_dir).encode())
            print(f"ntff profile: {nfiles} file(s) -> {output_dir}")

    mod = types.ModuleType("antenv.axon_hooks")
    mod.get_axon_ntff_profile_hook = lambda: _hook
    mod.set_axon_ntff_profile_hook = lambda h: None
    sys.modules["antenv.axon_hooks"] = mod
    import concourse.bass_utils as _bu
    _bu.upload_artifacts = lambda tmpdir: "local://" + str(tmpdir)


def _prep_and_run(x, src, dst, batch, W1, b1, W2, b2, Wfc, bfc, n, g_graphs):
    global LAST_EXEC_S, LAST_PREP_S, LAST_COMPILE_S
    t0 = time.perf_counter()
    npc = n // NCORES
    npcp = _ceil(npc, 128) * 128
    nb = npcp // 128

    deg = (np.bincount(dst, minlength=n) + 1.0).astype(np.float32)
    dinv = (1.0 / np.sqrt(deg)).astype(np.float32)

    sched, gidx1, gidx2, slot_cols = _schedule(src, dst, npc, nb)
    nt = sched.nt
    maxct = int(sched.ct.max())
    ninst_pad = max(K, _ceil(sched.ninst, K) * K)
    if os.environ.get("K_VERBOSE"):
        print(f"schedule: nt={nt} tiles, ninst={sched.ninst}, maxct={maxct}, "
              f"padded_idx={sched.ntot}, "
              f"real_edges/core={len(dst) / NCORES:.0f}, "
              f"pad_overhead={sched.ntot * NCORES / len(dst) - 1:.1%}")

    idx1_maps = _wrap_idx(sched, gidx1)
    idx2_maps = _wrap_idx(sched, gidx2)

    slots_pad = np.full((NCORES, 128, ninst_pad), -1.0, np.float32)
    slots_pad[:, :, : sched.ninst] = slot_cols

    dinv_pad = np.ones((NCORES, npcp), np.float32)
    batch_pad = np.full((NCORES, npcp), -1.0, np.float32)
    xs_pad = np.zeros((NCORES, npcp, F), ml_dtypes.bfloat16)
    xs = x * dinv[:, None]
    for c in range(NCORES):
        lo, hi = c * npc, (c + 1) * npc
        dinv_pad[c, :npc] = dinv[lo:hi]
        batch_pad[c, :npc] = batch[lo:hi].astype(np.float32)
        xs_pad[c, :npc] = xs[lo:hi].astype(ml_dtypes.bfloat16)
    dinv_cols = dinv_pad.reshape(NCORES, nb, 128).transpose(0, 2, 1).copy()
    batch_cols = batch_pad.reshape(NCORES, nb, 128).transpose(0, 2, 1).copy()

    # full x table in tbl1 (quarter-core-major) layout; zero gaps included
    xtbl = np.zeros((ALLOC_ROWS, F), ml_dtypes.bfloat16)
    for c in range(NCORES):
        for q in range(QN):
            l0, rpc = T1_QL0[q], T1_QRPC[q]
            xtbl[q * QROWS + c * rpc : q * QROWS + (c + 1) * rpc] = \
                xs_pad[c, l0 : l0 + rpc]

    iota = np.broadcast_to(np.arange(128), (128, 128)).astype(ml_dtypes.bfloat16)
    ident = np.eye(128, dtype=ml_dtypes.bfloat16)
    b1bc = np.broadcast_to(b1, (128, F)).astype(np.float32)
    w1b = W1.astype(ml_dtypes.bfloat16)
    w2b = W2.astype(ml_dtypes.bfloat16)

    LAST_PREP_S = time.perf_counter() - t0

    key = (n, g_graphs, npcp, sched.key())
    t0 = time.perf_counter()
    if key not in _BUILD_CACHE:
        _BUILD_CACHE.clear()
        _BUILD_CACHE[key] = _build_bass(
            npcp, nb, sched, ninst_pad, maxct, g_graphs
        )
    nc = _BUILD_CACHE[key]
    LAST_COMPILE_S = time.perf_counter() - t0

    in_maps = []
    for c in range(NCORES):
        in_maps.append(
            {
                "xtbl": xtbl,
                "xself": np.ascontiguousarray(xs_pad[c]),
                "idx1": np.ascontiguousarray(idx1_maps[c]),
                "idx2": np.ascontiguousarray(idx2_maps[c]),
                "slots": np.ascontiguousarray(
                    slots_pad[c].astype(ml_dtypes.bfloat16)
                ),
                "dinvc": np.ascontiguousarray(dinv_cols[c]),
                "batchc": np.ascontiguousarray(batch_cols[c]),
                "b1bc": b1bc,
                "iota": iota,
                "ident": ident,
                "w1": w1b,
                "w2": w2b,
            }
        )
    t0 = time.perf_counter()
    trace_dir = os.environ.get("K_TRACE_DIR")
    if trace_dir:
        _install_trace_hooks()
        res = run_bass_kernel_spmd(
            nc, in_maps, list(range(NCORES)), trace=True, tmpdir=trace_dir
        )
        globals()["LAST_RESULTS"] = res
    else:
        res = run_bass_kernel_spmd(nc, in_maps, list(range(NCORES)))
    LAST_EXEC_S = time.perf_counter() - t0

    pooled = np.zeros((g_graphs, H2), np.float64)
    for c in range(NCORES):
        pooled += res.results[c]["pooled"].astype(np.float64)
    cnt = np.bincount(batch, minlength=g_graphs).astype(np.float64)
    gmean = pooled / np.maximum(cnt, 1.0)[:, None]
    out = (gmean + b2.astype(np.float64)) @ Wfc.astype(np.float64) + bfc.astype(
        np.float64
    )
    return out.astype(np.float32)


def kernel(x, edge_index, batch, W1, b1, W2, b2, Wfc, bfc):
    x = np.asarray(x, dtype=np.float32)
    ei = np.asarray(edge_index)
    src = ei[0].astype(np.int64)
    dst = ei[1].astype(np.int64)
    bat = np.asarray(batch).astype(np.int64)
    n = x.shape[0]
    g_graphs = 64
    return _prep_and_run(
        x, src, dst, bat,
        np.asarray(W1, np.float32), np.asarray(b1, np.float32),
        np.asarray(W2, np.float32), np.asarray(b2, np.float32),
        np.asarray(Wfc, np.float32), np.asarray(bfc, np.float32),
        n, g_graphs,
    )
